# revision 23
# baseline (speedup 1.0000x reference)
"""Trainium2 Bass kernel for MultiHeadAttention + residual + LayerNorm.

Sharding: 8 cores = 4 batches x 2 query-halves. Each core computes, for its
(batch b, half q): K/V projections for the whole batch (2048 tokens, all 16
heads), Q projection for its 1024 query tokens, full attention for those
queries, the complete output projection, residual add and LayerNorm for its
token slice. Zero inter-core communication; the host concatenates the 8
[1024, 1024] slices.

v2 layout/schedule highlights (vs the phase-serial v1):
  - Projections are interleaved INTO the attention loop as "filler" units, so
    the tensor engine fills the bubbles where attn@V waits on exp, and the
    scalar engine starts exp ~150us earlier.
  - exp is split between ScalarE (activation, exact) and VectorE (Schraudolph
    int16 bit-trick: scores arrive pre-scaled by S=1024/ln2 -- folded into Wq
    on the host -- so DVE-exp is one tensor_scalar add with int16 output,
    bit-viewed as fp16). ~4/16 chunks go to DVE.
  - Softmax rows sum to 1 => bv folds into bo; bo folds into the residual xq
    on the host. V evacuation becomes a pure copy; no bias adds in phase 3.
  - Host rotates tokens so the query half is always x columns 0:1024;
    attention is token-permutation invariant, so K/V order just has to match.
  - LayerNorm uses rstd = exp(-0.5*ln(ssq/D + eps)): stays in the exp/ln
    activation-table set => no table swaps, LN can interleave with exp.
  - Scores are computed transposed (scoresT[t, s]) so exp(scoresT) feeds the
    attn@V matmul directly; K=64 score matmul pairs auto-row-tile to array
    rows 0:64 / 64:128 and run concurrently.
  - V stored per head as [t, 128] with columns 0..63 set to 1.0, so attn@V
    also produces the softmax denominator (rows 0..63) -> per-partition
    reciprocal, no cross-partition broadcast needed.
"""

import os
import sys

import numpy as np

for _p in ("/opt/trn_rl_repo", "/root/.axon_site/_ro/trn_rl_repo"):
    if os.path.isdir(_p) and _p not in sys.path:
        sys.path.insert(0, _p)

P = 128          # partitions
D = 1024         # model dim
EC = 8           # 128-chunks of the model dim
SQ = 1024        # query tokens per core
T = 2048         # kv tokens per core (one batch)
H = 16           # heads
HP = 8           # head pairs
DK = 64          # head dim
NT = 512         # matmul free-dim tile
N_CORES = 8
B, S = 4, 2048   # full problem

S_SCALE = 1024.0 / np.log(2.0)      # folded into Wq on host; psum score = S*x
EXP_B = 15360.0 - 58.6715           # f16 exp bias<<10, mean-zero log correction

_CACHE = {}
LAST_RESULTS = None


def _emit(tc, t):
    import concourse.bass as bass  # noqa: F401
    from concourse import mybir
    from contextlib import ExitStack

    nc = tc.nc
    f32 = mybir.dt.float32
    f16 = mybir.dt.float16
    i16 = mybir.dt.int16
    AF = mybir.ActivationFunctionType
    OP = mybir.AluOpType
    AX = mybir.AxisListType

    xT, xq, wqT, wkT, wvT, woT, bq, bk, consts, out = (
        t["xT"], t["xq"], t["wqT"], t["wkT"], t["wvT"], t["woT"],
        t["bq"], t["bk"], t["consts"], t["out"],
    )

    with ExitStack() as top:
        # DRAM staging as tile-pool tiles so Tile tracks RAW deps through them
        # (raw dram_tensors are not dependency-tracked). V is split by head
        # group so hp<4 attention doesn't wait on the second V projection.
        dstage = top.enter_context(tc.tile_pool(name="dstage", bufs=1,
                                                space="DRAM"))
        ktd = dstage.tile([D, T], f16, tag="ktd")
        qtd = dstage.tile([D, SQ], f16, tag="qtd")
        vd0 = dstage.tile([T, H // 2, DK], f16, tag="vd0")
        vd1 = dstage.tile([T, H // 2, DK], f16, tag="vd1")
        vdh = [vd0, vd1]
        persist = top.enter_context(tc.tile_pool(name="persist", bufs=1))
        cbc = persist.tile([P, 2 * D], f16, tag="cbc")       # gamma | beta
        ctxt_sb = persist.tile([P, EC, SQ], f16, tag="ctxt")
        ones1 = persist.tile([1, P], f16, tag="ones1")
        eps_t = persist.tile([P, 1], f32, tag="eps")
        ones_blk = persist.tile([P, (T // P) * 2 * DK], f16, tag="onesblk")
        csrow = persist.tile([1, 2 * D], f16, tag="csrow")
        rscr = persist.tile([DK, NT], f32, tag="rscr")

        wp = top.enter_context(tc.tile_pool(name="wqkv", bufs=1))
        wq_sb = wp.tile([P, EC, D], f16, tag="wq")
        wk_sb = wp.tile([P, EC, D], f16, tag="wk")
        wv_sb = wp.tile([P, EC, D], f16, tag="wv")
        wo_sb = wp.tile([P, EC, D], f16, tag="wo")
        bq_sb = wp.tile([P, EC], f32, tag="bq")
        bk_sb = wp.tile([P, EC], f32, tag="bk")
        xt_full = wp.tile([P, EC, T], f16, tag="xtf")

        nc.vector.memset(ones1[:], 1.0)
        nc.vector.memset(ones_blk[:], 1.0)
        nc.vector.memset(eps_t[:], 1e-5)
        nc.sync.dma_start(csrow[:], consts[:].rearrange("(o n) -> o n", o=1))

        # Input DMAs, chunked so the first projections can start early.
        def w_chunk(sb, drt, dc):
            nc.sync.dma_start(
                sb[:, :, dc * P:(dc + 1) * P],
                drt[:, dc * P:(dc + 1) * P].rearrange("(ec p) d -> p ec d", p=P))

        def x_chunk(c):
            nc.sync.dma_start(
                xt_full[:, :, c * NT:(c + 1) * NT],
                xT[:, c * NT:(c + 1) * NT].rearrange("(ec p) s -> p ec s", p=P))

        w_chunk(wq_sb, wqT, 0)
        x_chunk(0)
        x_chunk(1)
        nc.sync.dma_start(bq_sb[:], bq[:].rearrange("(ec p) -> p ec", p=P))
        w_chunk(wk_sb, wkT, 0)
        nc.sync.dma_start(bk_sb[:], bk[:].rearrange("(ec p) -> p ec", p=P))
        x_chunk(2)
        x_chunk(3)
        for dc in range(1, EC):
            w_chunk(wq_sb, wqT, dc)
            w_chunk(wk_sb, wkT, dc)
        nc.sync.dma_start(wv_sb[:, :, 0:NT],
                          wvT[:, 0:NT].rearrange("(ec p) d -> p ec d", p=P))
        nc.sync.dma_start(wv_sb[:, :, NT:D],
                          wvT[:, NT:D].rearrange("(ec p) d -> p ec d", p=P))

        ep = top.enter_context(tc.tile_pool(name="ev", bufs=3))
        pp = top.enter_context(tc.tile_pool(name="pp", bufs=2, space="PSUM"))

        # broadcast gamma|beta across partitions via ones-matmul
        for i in range(4):
            pt = pp.tile([P, NT], f32, tag="ps")
            nc.tensor.matmul(pt[:], lhsT=ones1[:],
                             rhs=csrow[:, i * NT:(i + 1) * NT],
                             start=True, stop=True)
            nc.vector.tensor_copy(cbc[:, i * NT:(i + 1) * NT], pt[:])

        # ---------------- projection units ----------------
        def q_unit(dc, st):
            ps = pp.tile([P, NT], f32, tag="ps")
            for ec in range(EC):
                nc.tensor.matmul(ps[:], lhsT=wq_sb[:, ec, dc * P:(dc + 1) * P],
                                 rhs=xt_full[:, ec, st * NT:(st + 1) * NT],
                                 start=(ec == 0), stop=(ec == EC - 1))
            qe = ep.tile([P, NT], f16, tag="ev")
            nc.vector.tensor_scalar_add(qe[:], ps[:], bq_sb[:, dc:dc + 1])
            nc.sync.dma_start(qtd[dc * P:(dc + 1) * P, st * NT:(st + 1) * NT], qe[:])

        def k_unit(dc, tt):
            ps = pp.tile([P, NT], f32, tag="ps")
            for ec in range(EC):
                nc.tensor.matmul(ps[:], lhsT=wk_sb[:, ec, dc * P:(dc + 1) * P],
                                 rhs=xt_full[:, ec, tt * NT:(tt + 1) * NT],
                                 start=(ec == 0), stop=(ec == EC - 1))
            ke = ep.tile([P, NT], f16, tag="ev")
            nc.vector.tensor_scalar_add(ke[:], ps[:], bk_sb[:, dc:dc + 1])
            nc.sync.dma_start(ktd[dc * P:(dc + 1) * P, tt * NT:(tt + 1) * NT], ke[:])

        def v_unit(dt, tcg):
            ps = pp.tile([P, NT], f32, tag="ps")
            for ec in range(EC):
                nc.tensor.matmul(ps[:], lhsT=xt_full[:, ec, tcg * P:(tcg + 1) * P],
                                 rhs=wv_sb[:, ec, dt * NT:(dt + 1) * NT],
                                 start=(ec == 0), stop=(ec == EC - 1))
            ve = ep.tile([P, NT], f16, tag="ev")
            nc.vector.tensor_copy(ve[:], ps[:])
            nc.sync.dma_start(
                vdh[dt][tcg * P:(tcg + 1) * P, :, :],
                ve[:].rearrange("p (h k) -> p h k", k=DK))

        def g_units(dc):
            return [lambda tt=tt: k_unit(dc, tt) for tt in range(4)] + \
                   [lambda st=st: q_unit(dc, st) for st in range(2)]

        # Prologue: everything attention hp=0 and hp=1 needs.
        for u in g_units(0):
            u()
        for tcg in range(T // P):
            v_unit(0, tcg)
        for u in g_units(1):
            u()
        for dc in range(EC):
            w_chunk(wo_sb, woT, dc)

        # Filler queue for attention bubbles (2 units per fill point). Each
        # unit carries the hp by which it must have been EMITTED (so the
        # consuming load carries a dependency on it); drain_until enforces it.
        filler = []
        for dc in (2, 3):
            filler += [(dc, u) for u in g_units(dc)]
        filler += [(4, (lambda tcg=tcg: v_unit(1, tcg))) for tcg in range(T // P)]
        for dc in (4, 5, 6, 7):
            filler += [(dc, u) for u in g_units(dc)]

        def pop_filler(n=2):
            for _ in range(min(n, len(filler))):
                filler.pop(0)[1]()

        def drain_until(hp):
            while filler and filler[0][0] <= hp:
                filler.pop(0)[1]()

        # ---------------- phase 3 helpers (out proj + residual + LN) --------
        xqp = top.enter_context(tc.tile_pool(name="xqp", bufs=2))
        yp = top.enter_context(tc.tile_pool(name="yp", bufs=2))
        scr = top.enter_context(tc.tile_pool(name="scr", bufs=2))
        stp = top.enter_context(tc.tile_pool(name="stats", bufs=8))
        outp = top.enter_context(tc.tile_pool(name="outp", bufs=2))

        def out_sc(sc):
            y = yp.tile([P, D], f32, tag="y")
            xqt = xqp.tile([P, D], f16, tag="xq")
            nc.sync.dma_start(xqt[:], xq[sc * P:(sc + 1) * P, :])
            for et in range(D // NT):
                ps = pp.tile([P, NT], f32, tag="ps")
                for dc in range(EC):
                    nc.tensor.matmul(ps[:], lhsT=ctxt_sb[:, dc, sc * P:(sc + 1) * P],
                                     rhs=wo_sb[:, dc, et * NT:(et + 1) * NT],
                                     start=(dc == 0), stop=(dc == EC - 1))
                nc.vector.tensor_tensor(y[:, et * NT:(et + 1) * NT], ps[:],
                                        xqt[:, et * NT:(et + 1) * NT], OP.add)
            # LayerNorm over the free dim (sqrt-free: exp/ln table set only)
            nmean = stp.tile([P, 1], f32, tag="st")
            nc.vector.tensor_reduce(nmean[:], y[:], AX.X, OP.add, negate=True)
            nc.vector.tensor_scalar_mul(nmean[:], nmean[:], 1.0 / D)
            cent = scr.tile([P, D], f32, tag="cent")
            nc.vector.tensor_scalar_add(cent[:], y[:], nmean[:])
            o = outp.tile([P, D], f32, tag="o")
            ssq = stp.tile([P, 1], f32, tag="st")
            # Square's elementwise output is scratch (we only need accum_out);
            # park it in o, which stt fully overwrites below.
            nc.scalar.activation(o[:], cent[:], AF.Square, accum_out=ssq[:])
            lnv = stp.tile([P, 1], f32, tag="st")
            nc.scalar.activation(lnv[:], ssq[:], AF.Ln, bias=eps_t[:], scale=1.0 / D)
            rstd = stp.tile([P, 1], f32, tag="st")
            nc.scalar.activation(rstd[:], lnv[:], AF.Exp, scale=-0.5)
            nc.vector.scalar_tensor_tensor(o[:], in0=cent[:], scalar=rstd[:],
                                           in1=cbc[:, 0:D],
                                           op0=OP.mult, op1=OP.mult)
            nc.vector.tensor_tensor(o[:], o[:], cbc[:, D:2 * D], OP.add)
            nc.sync.dma_start(out[sc * P:(sc + 1) * P, :], o[:])

        # ---------------- attention ----------------
        ktp = top.enter_context(tc.tile_pool(name="ktp", bufs=2))
        qtp = top.enter_context(tc.tile_pool(name="qtp", bufs=2))
        vp = top.enter_context(tc.tile_pool(name="vp", bufs=1))
        ptp = top.enter_context(tc.tile_pool(name="ptp", bufs=10))
        rcp = top.enter_context(tc.tile_pool(name="rcp", bufs=2))
        sps = top.enter_context(tc.tile_pool(name="sps", bufs=2, space="PSUM"))
        cps = top.enter_context(tc.tile_pool(name="cps", bufs=2, space="PSUM"))

        DVE_EXP = (2, 6)  # chunks per 8-half routed to VectorE fast-exp

        for hp in range(HP):
            drain_until(hp)
            kt_t = ktp.tile([P, T], f16, tag="kt")
            nc.sync.dma_start(kt_t[:], ktd[hp * P:(hp + 1) * P, :])
            qt_t = qtp.tile([P, SQ], f16, tag="qt")
            nc.sync.dma_start(qt_t[:], qtd[hp * P:(hp + 1) * P, :])
            v_t = vp.tile([P, T // P, 2, P], f16, tag="v")
            for h2 in (0, 1):
                hloc = (2 * hp + h2) % (H // 2)
                nc.sync.dma_start(
                    v_t[:, :, h2, DK:P],
                    vdh[hp // 4][:].rearrange(
                        "(tc p) h k -> p tc h k", p=P)[:, :, hloc, :])
            nc.vector.tensor_copy(
                v_t[:, :, :, 0:DK],
                ones_blk[:].rearrange("p (a b c) -> p a b c", b=2, c=DK))

            for st in range(SQ // NT):
                c0 = cps.tile([P, NT], f32, tag="cps")
                c1 = cps.tile([P, NT], f32, tag="cps")
                for half in (0, 1):
                    pts = []
                    for tj in range(T // P // 2):
                        tcc = half * 8 + tj
                        sp = sps.tile([P, 2 * NT], f32, tag="sps")
                        nc.tensor.matmul(sp[:, 0:NT],
                                         lhsT=kt_t[0:DK, tcc * P:(tcc + 1) * P],
                                         rhs=qt_t[0:DK, st * NT:(st + 1) * NT],
                                         start=True, stop=True)
                        nc.tensor.matmul(sp[:, NT:2 * NT],
                                         lhsT=kt_t[DK:P, tcc * P:(tcc + 1) * P],
                                         rhs=qt_t[DK:P, st * NT:(st + 1) * NT],
                                         start=True, stop=True)
                        pt = ptp.tile([P, 2 * NT], f16, tag="pt")
                        if tj in DVE_EXP:
                            nc.vector.tensor_scalar(
                                out=pt[:].bitcast(i16), in0=sp[:],
                                scalar1=float(EXP_B), scalar2=None, op0=OP.add)
                        else:
                            nc.scalar.activation(pt[:], sp[:], AF.Exp,
                                                 scale=float(1.0 / S_SCALE))
                        pts.append((tcc, pt))
                    if hp == HP - 1 and st == 1:
                        for sc in (2 * half, 2 * half + 1):
                            out_sc(sc)
                    else:
                        pop_filler(2)
                    for tcc, pt in pts:
                        nc.tensor.matmul(c0[:], lhsT=v_t[:, tcc, 0, :],
                                         rhs=pt[:, 0:NT],
                                         start=(tcc == 0), stop=(tcc == T // P - 1))
                        nc.tensor.matmul(c1[:], lhsT=v_t[:, tcc, 1, :],
                                         rhs=pt[:, NT:2 * NT],
                                         start=(tcc == 0), stop=(tcc == T // P - 1))
                for h2, cc in ((0, c0), (1, c1)):
                    rec = rcp.tile([DK, NT], f32, tag="rec")
                    nc.vector.reciprocal_approx_accurate(rec[:], cc[0:DK, :], rscr[:])
                    nc.vector.tensor_tensor(
                        ctxt_sb[h2 * DK:(h2 + 1) * DK, hp, st * NT:(st + 1) * NT],
                        cc[DK:P, :], rec[:], OP.mult)

        while filler:
            pop_filler(4)
        for sc in range(4, SQ // P):
            out_sc(sc)


def _build():
    if "nc" in _CACHE:
        return _CACHE["nc"]
    from concourse import bacc, mybir
    import concourse.tile as tile

    f32 = mybir.dt.float32
    nc = bacc.Bacc("TRN2", target_bir_lowering=False, debug=False)
    t = {}
    f16 = mybir.dt.float16
    t["xT"] = nc.dram_tensor("xT", [D, T], f16, kind="ExternalInput")
    t["xq"] = nc.dram_tensor("xq", [SQ, D], f16, kind="ExternalInput")
    t["wqT"] = nc.dram_tensor("wqT", [D, D], f16, kind="ExternalInput")
    t["wkT"] = nc.dram_tensor("wkT", [D, D], f16, kind="ExternalInput")
    t["wvT"] = nc.dram_tensor("wvT", [D, D], f16, kind="ExternalInput")
    t["woT"] = nc.dram_tensor("woT", [D, D], f16, kind="ExternalInput")
    t["bq"] = nc.dram_tensor("bq", [D], f32, kind="ExternalInput")
    t["bk"] = nc.dram_tensor("bk", [D], f32, kind="ExternalInput")
    t["consts"] = nc.dram_tensor("consts", [2 * D], f16, kind="ExternalInput")
    t["out"] = nc.dram_tensor("out", [SQ, D], f32, kind="ExternalOutput")

    with tile.TileContext(nc) as tc:
        _emit(tc, t)
    nc.compile()
    _CACHE["nc"] = nc
    return nc


def _prep_inputs(x, Wq, bq, Wk, bk, Wv, bv, Wo, bo, ln_gamma, ln_beta):
    """Host-side sharding/layout prep. Returns per-core input maps."""
    f = np.float32
    h = np.float16
    x = np.asarray(x, f)
    qs = float(S_SCALE) / 8.0
    wqT = np.ascontiguousarray((np.asarray(Wq, f).T * qs).astype(h))
    wkT = np.ascontiguousarray(np.asarray(Wk, f).T.astype(h))
    wvT = np.ascontiguousarray(np.asarray(Wv, f).T.astype(h))
    woT = np.ascontiguousarray(np.asarray(Wo, f).T.astype(h))
    bq_s = np.asarray(bq, f) * qs
    # softmax rows sum to 1 => attn@(V + bv) = attn@V + bv; fold into bo,
    # then fold bo into the residual input xq.
    bo2 = np.asarray(Wo, f) @ np.asarray(bv, f) + np.asarray(bo, f)
    consts = np.concatenate(
        [np.asarray(ln_gamma, f), np.asarray(ln_beta, f)]).astype(h)
    in_maps = []
    for c in range(N_CORES):
        b, half = c // 2, c % 2
        xb = x[b]                                        # [2048, 1024]
        xslice = xb[half * SQ:(half + 1) * SQ]           # [1024, 1024]
        # rotate tokens so the query half is always columns 0:SQ of xT
        if half == 0:
            xrot = xb
        else:
            xrot = np.concatenate([xb[SQ:], xb[:SQ]], axis=0)
        in_maps.append({
            "xT": np.ascontiguousarray(xrot.T).astype(h),
            "xq": np.ascontiguousarray(xslice + bo2[None, :]).astype(h),
            "wqT": wqT, "wkT": wkT, "wvT": wvT, "woT": woT,
            "bq": bq_s, "bk": np.asarray(bk, f),
            "consts": consts,
        })
    return in_maps


def _ensure_axon_hooks_shim():
    """This image's `antenv` lacks the `axon_hooks` registry module that
    `run_bass_kernel_spmd(trace=True)` imports. Provide it (hook installed
    from the boot .so when available, else None -> tracing degrades
    gracefully instead of raising ImportError)."""
    import importlib
    import types

    try:
        importlib.import_module("antenv.axon_hooks")
        return
    except ImportError:
        pass
    mod = types.ModuleType("antenv.axon_hooks")
    _state = {"hook": None}
    mod.set_axon_ntff_profile_hook = lambda h: _state.update(hook=h)
    mod.get_axon_ntff_profile_hook = lambda: _state["hook"]
    sys.modules["antenv.axon_hooks"] = mod
    try:
        import antenv
        antenv.axon_hooks = mod
    except Exception:
        pass
    try:
        from trn_agent_boot.trn_boot import _ntff_profile_via_ctypes
        so = "/opt/axon/libaxon_pjrt.so"
        if os.path.exists(so):
            mod.set_axon_ntff_profile_hook(_ntff_profile_via_ctypes(so))
    except Exception:
        pass


def kernel(**inputs):
    global LAST_RESULTS
    _ensure_axon_hooks_shim()
    from concourse.bass_utils import run_bass_kernel_spmd

    nc = _build()
    in_maps = _prep_inputs(**inputs)
    trace = bool(os.environ.get("MHA_TRACE"))
    res = run_bass_kernel_spmd(nc, in_maps, core_ids=list(range(N_CORES)),
                               trace=trace)
    LAST_RESULTS = res
    out = np.empty((B, S, D), np.float32)
    for c in range(N_CORES):
        b, half = c // 2, c % 2
        out[b, half * SQ:(half + 1) * SQ, :] = res.results[c]["out"]
    return out


if __name__ == "__main__":
    from reference import setup_inputs, reference
    import jax
    with jax.default_device(jax.devices("cpu")[0]):
        inp = {k: np.asarray(v) for k, v in setup_inputs().items()}
        exp = np.asarray(reference(**inp))
    act = kernel(**inp)
    err = np.linalg.norm(act - exp) / np.linalg.norm(exp)
    print("Relative error:", err)
